# revision 1
# baseline (speedup 1.0000x reference)
"""AdaptiveFeaturePropagation Trainium2 kernel (8 NeuronCores, SPMD).

Sharding: 8 cores = (batch 4) x (H halves 2); halos replicated host-side, no
collectives. Per core (shard rows [s, s+32)):
  conv1 (3x3, 1024->256, applied to cur & key): bf16 matmuls, contraction
    over Cin in 128-chunks x 9 taps accumulated in PSUM. Inputs arrive
    bf16 in layout [128, rows, chunk*W] so each 7-row block is ONE DMA
    with 9.5KB contiguous runs per partition. w1 streams per-chunk so the
    first matmul starts ~5us in.
  conv2 (3x3, 512->256) in 8-row blocks (N=512) and conv3 (1x1, 256->81).
  conv3 is computed "swapped" (x3 pixel-block as the stationary operand) so
  kernel logits land pixel-major [128 pix, 81]; softmax = Relu+Exp on ACT with
  accum_out giving the denominator; normalization folded into the SVC drain.
  Spatially-variant 9x9 conv: banded-matrix matmul
    out[pix, c] = sum_g band_g[s,j].T @ highT[g][s, c]
  with the band built by ONE DMA scatter per row-pair into a DRAM image at
  addr 129*p+128*k (zero margins), loaded back contiguously in one 4D-AP
  DMA, and a static 0/1 mask zeroing aliased tap slots.
  Pipeline is interleaved at row-pair granularity (conv3(t) ~4 pairs ahead
  of svc(t)) to spread the scatter descriptor load; DMA issuance is spread
  across engine queues (inputs/weights/highT: scalar, band reload: sync,
  scatter+zero-fill: gpsimd, output: vector).
Output is written pixel-major [2048, 1024] bf16 per core; host transposes
and upcasts.
"""

import sys

sys.path.insert(0, "/opt/trn_rl_repo")

import numpy as np
import ml_dtypes

import concourse.bacc as bacc
import concourse.mybir as mybir
from concourse.bass_types import AP
from concourse.tile import TileContext
from concourse.bass_utils import run_bass_kernel_spmd

F32 = mybir.dt.float32
BF16 = mybir.dt.bfloat16
BF = ml_dtypes.bfloat16
AF = mybir.ActivationFunctionType

# ---------------- configuration ----------------


class Cfg:
    B = 4
    HALVES = 2
    H = 64
    W = 64
    C_IN = 1024  # conv1 input channels
    CO1 = 256  # conv1 output channels (per branch)
    C2 = 512  # conv2 input channels
    CO2 = 256  # conv2 output channels
    K81 = 81
    CH = 1024  # high-feature channels

    OUT_ROWS = 32  # output rows per shard
    # conv1 output rows = OUT_ROWS + 2 (halo +-1), input rows = OUT_ROWS + 4
    C1_BLOCKS = ((0, 4), (4, 8), (12, 8), (20, 8), (28, 6))
    C2_ROWS = 8  # conv2 block rows (N = 512)

    @property
    def X2_ROWS(self):
        return self.OUT_ROWS + 2

    @property
    def IN_ROWS(self):
        return self.OUT_ROWS + 4

    @property
    def HT_ROWS(self):
        return self.OUT_ROWS + 8

    @property
    def PAIRS(self):
        return self.OUT_ROWS // 2

    @property
    def WP(self):
        return self.W + 2

    @property
    def WH(self):
        return self.W + 8

    # band image geometry: kt stored at addr = 91*p + k (k-pad to 91);
    # band slot (s,g,rr,w) reads addr = 5815*rr + 90*w + 9*g + s, loaded as
    # a transposed-DMA with in_ = [[9, 640], [1, 128]] per rr half.
    XST = 91  # kt image p stride
    XRR = 5815  # rr half offset = 64*91 - 9

    @property
    def XSZ(self):  # per-pair image size (>= 5815 + 9*639 + 127 + 1)
        return 11776


CFG = Cfg()

# ---------------- graph builder ----------------


def build_graph(cfg):
    nc = bacc.Bacc(None, target_bir_lowering=False)
    W = cfg.W
    n_cin_ch = cfg.C_IN // 128
    n_c2_ch = cfg.C2 // 128
    n_co1_h = cfg.CO1 // 128
    n_co2_h = cfg.CO2 // 128
    n_cc = cfg.CH // 512  # SVC c-chunks
    PAIRS = cfg.PAIRS
    CWP = n_cin_ch * cfg.WP  # 528: chunk*W row pitch
    NJ = (cfg.HT_ROWS + 1) // 2  # paired highT tiles

    cur_e = nc.declare_dram_parameter(
        "cur", [128, cfg.IN_ROWS, CWP], BF16, isOutput=False
    )
    key_e = nc.declare_dram_parameter(
        "key", [128, cfg.IN_ROWS, CWP], BF16, isOutput=False
    )
    high_e = nc.declare_dram_parameter(
        "highT", [cfg.HT_ROWS, cfg.WH, cfg.CH], BF16, isOutput=False
    )
    # w1 holds Winograd-F(2,3)-transformed weights:
    # [128ci, chunk, dy(3)*m(4)*co] so per-chunk loads are contiguous
    w1_e = nc.declare_dram_parameter(
        "w1", [128, n_cin_ch, 12 * cfg.CO1], BF16, isOutput=False
    )
    # w2 holds Winograd-F(2,3)-transformed weights [128ci, chunk, dy*m*co]
    w2_e = nc.declare_dram_parameter(
        "w2", [128, n_c2_ch, 12 * cfg.CO2], BF16, isOutput=False
    )
    w3_e = nc.declare_dram_parameter(
        "w3", [128, cfg.CO2 // 128, cfg.K81], BF16, isOutput=False
    )
    b1_e = nc.declare_dram_parameter("b1", [128, n_co1_h], F32, isOutput=False)
    b2_e = nc.declare_dram_parameter("b2", [128, n_co2_h], F32, isOutput=False)
    b3_e = nc.declare_dram_parameter("b3", [128, cfg.K81], F32, isOutput=False)
    hmask_e = nc.declare_dram_parameter("hmask", [128, 2], F32, isOutput=False)
    bmask_e = nc.declare_dram_parameter(
        "bmask", [cfg.WH, 10 * 2 * W], BF16, isOutput=False
    )
    out_e = nc.declare_dram_parameter(
        "out", [cfg.OUT_ROWS * W, cfg.CH], BF16, isOutput=True
    )

    ximgs = [
        nc.dram_tensor(f"ximg{t}", [cfg.XSZ], BF16) for t in range(PAIRS)
    ]

    with TileContext(nc) as tc:
        with (
            tc.tile_pool(name="const", bufs=1) as cpool,
            tc.tile_pool(name="feat", bufs=1) as fpool,
            tc.tile_pool(name="c1in", bufs=3) as inpool,
            tc.tile_pool(name="dw", bufs=3) as dpool,
            tc.tile_pool(name="wg", bufs=4) as wpool,
            tc.tile_pool(name="ht", bufs=6) as htpool,
            tc.tile_pool(name="band", bufs=6) as bandpool,
            tc.tile_pool(name="small", bufs=8) as spool,
            tc.tile_pool(name="rd", bufs=16) as rdpool,
            tc.tile_pool(name="ob", bufs=3) as obpool,
            tc.tile_pool(name="ps", bufs=8, space="PSUM") as pspool,
        ):
            # ---- persistent constants ----
            w1sb = cpool.tile([128, n_cin_ch * 12 * cfg.CO1], BF16)
            b1sb = cpool.tile([128, n_co1_h], F32)
            hmsb = cpool.tile([128, 2], F32)
            w2sb = cpool.tile([128, n_c2_ch * 12 * cfg.CO2], BF16)
            w3sb = cpool.tile([128, (cfg.CO2 // 128) * cfg.K81], BF16)
            b2sb = cpool.tile([128, n_co2_h], F32)
            b3sb = cpool.tile([128, cfg.K81], F32)
            bmsb = cpool.tile([cfg.WH, 10 * 2 * W], BF16)
            zt = cpool.tile([128, cfg.XSZ // 128], BF16)  # 92 cols

            W1CH = 12 * cfg.CO1  # per-chunk w1 stride in sbuf

            def load_w1_chunk(ch):
                nc.scalar.dma_start(
                    out=w1sb[:, ch * W1CH : (ch + 1) * W1CH],
                    in_=w1_e[:, ch, :],
                )

            nc.vector.memset(zt[:], 0.0)

            def emit_zero_fills():
                # zero-fill band images (gpsimd queue; deferred past the
                # startup DMA burst, needed before the first kt write)
                for t in range(PAIRS):
                    dz = AP(
                        ximgs[t],
                        0,
                        [[cfg.XSZ // 128, 128], [1, cfg.XSZ // 128]],
                    )
                    nc.gpsimd.dma_start(out=dz, in_=zt[:])

            def emit_deferred_consts():
                emit_zero_fills()
                nc.scalar.dma_start(out=w2sb[:], in_=w2_e[:, :, :])
                nc.scalar.dma_start(out=w3sb[:], in_=w3_e[:, :, :])
                nc.scalar.dma_start(out=b2sb[:], in_=b2_e[:, :])
                nc.scalar.dma_start(out=b3sb[:], in_=b3_e[:, :])
                nc.scalar.dma_start(out=bmsb[:], in_=bmask_e[:, :])

            # x2 (conv1 out, conv2 in), bf16, padded cols; x3 (conv2 out)
            x2c = []
            for i in range(2 * n_co1_h):
                t_ = fpool.tile([128, cfg.X2_ROWS * cfg.WP], BF16, tag=f"x2_{i}")
                nc.vector.memset(t_[:], 0.0)
                x2c.append(t_)
            x3c = []
            for i in range(n_co2_h):
                t_ = fpool.tile([128, cfg.OUT_ROWS * W], BF16, tag=f"x3_{i}")
                x3c.append(t_)

            # paired highT ring: tile j holds groups (2j, 2j+1)
            ht2 = {}

            def need_ht2(j):
                if j >= NJ:
                    return
                if j not in ht2:
                    h_ = htpool.tile([cfg.WH, 2 * cfg.CH], BF16, tag="ht")
                    src = AP(
                        high_e,
                        2 * j * cfg.WH * cfg.CH,
                        [
                            [cfg.CH, cfg.WH],
                            [cfg.WH * cfg.CH, 2],
                            [1, cfg.CH],
                        ],
                    )
                    dst = h_[:, :].rearrange("s (g c) -> s g c", g=2)
                    nc.scalar.dma_start(out=dst, in_=src)
                    ht2[j] = h_

            def ht_slice(g, cc):
                j = g // 2
                c0 = (g % 2) * cfg.CH + 512 * cc
                return ht2[j][:, c0 : c0 + 512]

            # ---- conv1 (cur, key) -> x2, Winograd F(2,3) along W ----
            # y[2j]   = m1 + m2 + m3,  y[2j+1] = m2 - m3 - m4 where
            # m_i = D_i . gw_i with D1 = d0-d2, D2 = d1+d2, D3 = d2-d1,
            # D4 = d1-d3 over padded cols (2j, 2j+1, 2j+2, 2j+3); vertical
            # taps stay direct (dy row shifts of the shared D planes).
            def emit_c1_block(bi, after_inputs=None):
                o0, nout = cfg.C1_BLOCKS[bi]
                nin = nout + 2
                J = W // 2
                its = []
                for inp_e in (cur_e, key_e):
                    it = inpool.tile([128, nin * CWP], BF16, tag="c1in")
                    nc.scalar.dma_start(out=it[:], in_=inp_e[:, o0 : o0 + nin, :])
                    its.append(it)
                if after_inputs is not None:
                    after_inputs()
                for ii, it in enumerate(its):
                    itv5 = it[:, :].rearrange(
                        "p (r c two w2) -> p r c two w2",
                        c=n_cin_ch, two=2, w2=cfg.WP // 2,
                    )
                    psm = [
                        [
                            pspool.tile(
                                [128, nout * J], F32, tag="ps",
                                name=f"psw_{o0}_{ii}_{m_}_{h_}",
                            )
                            for h_ in range(n_co1_h)
                        ]
                        for m_ in range(4)
                    ]
                    for ch in range(n_cin_ch):
                        dt = dpool.tile([128, nin * 4 * J], BF16, tag="d")
                        dtv = dt[:, :].rearrange("p (r m j) -> p r m j", m=4, j=J)
                        s0 = itv5[:, :, ch, 0, 0:J]
                        s1 = itv5[:, :, ch, 1, 0:J]
                        s2 = itv5[:, :, ch, 0, 1 : J + 1]
                        s3 = itv5[:, :, ch, 1, 1 : J + 1]
                        nc.vector.tensor_sub(dtv[:, :, 0, :], s0, s2)
                        nc.vector.tensor_add(dtv[:, :, 1, :], s1, s2)
                        nc.vector.tensor_sub(dtv[:, :, 2, :], s2, s1)
                        nc.vector.tensor_sub(dtv[:, :, 3, :], s1, s3)
                        for dy in range(3):
                            for m_ in range(4):
                                rhs = dtv[:, dy : dy + nout, m_, :]
                                for hf in range(n_co1_h):
                                    c0 = ch * W1CH + (dy * 4 + m_) * cfg.CO1 + 128 * hf
                                    nc.tensor.matmul(
                                        psm[m_][hf][:, :],
                                        w1sb[:, c0 : c0 + 128],
                                        rhs,
                                        start=(ch == 0 and dy == 0),
                                        stop=(ch == n_cin_ch - 1 and dy == 2),
                                    )
                    for hf in range(n_co1_h):
                        p1, p2, p3, p4 = (psm[m_][hf][:, :] for m_ in range(4))
                        # TensorTensor reads at most one PSUM input: stage m2
                        t2 = wpool.tile([128, nout * J], F32, tag="w2c")
                        ta = wpool.tile([128, nout * J], F32, tag="wya")
                        tb = wpool.tile([128, nout * J], F32, tag="wyb")
                        nc.vector.tensor_copy(t2[:, :], p2)
                        nc.vector.tensor_add(ta[:, :], p1, t2[:, :])
                        nc.vector.tensor_add(ta[:, :], ta[:, :], p3)
                        nc.vector.tensor_sub(tb[:, :], t2[:, :], p3)
                        nc.vector.tensor_sub(tb[:, :], tb[:, :], p4)
                        x2v5 = x2c[ii * n_co1_h + hf][:, :].rearrange(
                            "p (r two w2) -> p r two w2", two=2, w2=cfg.WP // 2
                        )
                        dst_even = x2v5[:, o0 : o0 + nout, 1, 0:J]
                        dst_odd = x2v5[:, o0 : o0 + nout, 0, 1 : J + 1]
                        nc.scalar.activation(
                            dst_even, ta[:, :], AF.Relu, bias=b1sb[:, hf : hf + 1]
                        )
                        nc.scalar.activation(
                            dst_odd, tb[:, :], AF.Relu, bias=b1sb[:, hf : hf + 1]
                        )

            # halo row masking (rows 0 and X2_ROWS-1 of x2)
            lr = cfg.X2_ROWS - 1

            def emit_mask_top():
                for i in range(2 * n_co1_h):
                    nc.vector.tensor_scalar_mul(
                        x2c[i][:, 0 : cfg.WP], x2c[i][:, 0 : cfg.WP], hmsb[:, 0:1]
                    )

            def emit_mask_bot():
                for i in range(2 * n_co1_h):
                    nc.vector.tensor_scalar_mul(
                        x2c[i][:, lr * cfg.WP : (lr + 1) * cfg.WP],
                        x2c[i][:, lr * cfg.WP : (lr + 1) * cfg.WP],
                        hmsb[:, 1:2],
                    )

            # ---- conv2 -> x3 (8-row blocks), Winograd F(2,3) along W ----
            W2CH = 12 * cfg.CO2

            def emit_c2_block(b):
                nr = cfg.C2_ROWS
                nin2 = nr + 2
                J = W // 2
                r0 = nr * b
                psm = [
                    [
                        pspool.tile(
                            [128, nr * J], F32, tag="ps",
                            name=f"ps2w_{b}_{m_}_{h_}",
                        )
                        for h_ in range(n_co2_h)
                    ]
                    for m_ in range(4)
                ]
                for ch in range(n_c2_ch):
                    x2v = x2c[ch][:, :].rearrange(
                        "p (r two w2) -> p r two w2", two=2, w2=cfg.WP // 2
                    )
                    dt = dpool.tile([128, nin2 * 4 * J], BF16, tag="d")
                    dtv = dt[:, :].rearrange("p (r m j) -> p r m j", m=4, j=J)
                    s0 = x2v[:, r0 : r0 + nin2, 0, 0:J]
                    s1 = x2v[:, r0 : r0 + nin2, 1, 0:J]
                    s2 = x2v[:, r0 : r0 + nin2, 0, 1 : J + 1]
                    s3 = x2v[:, r0 : r0 + nin2, 1, 1 : J + 1]
                    nc.vector.tensor_sub(dtv[:, :, 0, :], s0, s2)
                    nc.vector.tensor_add(dtv[:, :, 1, :], s1, s2)
                    nc.vector.tensor_sub(dtv[:, :, 2, :], s2, s1)
                    nc.vector.tensor_sub(dtv[:, :, 3, :], s1, s3)
                    for dy in range(3):
                        for m_ in range(4):
                            rhs = dtv[:, dy : dy + nr, m_, :]
                            for hf in range(n_co2_h):
                                c0 = ch * W2CH + (dy * 4 + m_) * cfg.CO2 + 128 * hf
                                nc.tensor.matmul(
                                    psm[m_][hf][:, :],
                                    w2sb[:, c0 : c0 + 128],
                                    rhs,
                                    start=(ch == 0 and dy == 0),
                                    stop=(ch == n_c2_ch - 1 and dy == 2),
                                )
                for hf in range(n_co2_h):
                    p1, p2, p3, p4 = (psm[m_][hf][:, :] for m_ in range(4))
                    t2 = wpool.tile([128, nr * J], F32, tag="w2c")
                    ta = wpool.tile([128, nr * J], F32, tag="wya")
                    tb = wpool.tile([128, nr * J], F32, tag="wyb")
                    nc.vector.tensor_copy(t2[:, :], p2)
                    nc.vector.tensor_add(ta[:, :], p1, t2[:, :])
                    nc.vector.tensor_add(ta[:, :], ta[:, :], p3)
                    nc.vector.tensor_sub(tb[:, :], t2[:, :], p3)
                    nc.vector.tensor_sub(tb[:, :], tb[:, :], p4)
                    x3v = x3c[hf][:, :].rearrange(
                        "p (r w2 two) -> p r w2 two", w2=J, two=2
                    )
                    nc.scalar.activation(
                        x3v[:, r0 : r0 + nr, :, 0], ta[:, :],
                        AF.Relu, bias=b2sb[:, hf : hf + 1],
                    )
                    nc.scalar.activation(
                        x3v[:, r0 : r0 + nr, :, 1], tb[:, :],
                        AF.Relu, bias=b2sb[:, hf : hf + 1],
                    )

            # ---- per row-pair: conv3 + softmax + band scatter/reload ----
            def emit_conv3(t):
                need_ht2(t + 4)  # prefetch the tile svc(t) will need last
                ps3 = pspool.tile([128, cfg.K81], F32, tag="ps")
                for ch in range(cfg.CO2 // 128):
                    nc.tensor.matmul(
                        ps3[:, :],
                        x3c[ch][:, t * 128 : (t + 1) * 128],
                        w3sb[:, ch * cfg.K81 : (ch + 1) * cfg.K81],
                        start=(ch == 0),
                        stop=(ch == cfg.CO2 // 128 - 1),
                    )
                t81 = spool.tile([128, cfg.K81], F32, tag="t81")
                nc.vector.tensor_add(t81[:], ps3[:, :], b3sb[:])
                nc.scalar.activation(t81[:], t81[:], AF.Relu)
                kt = spool.tile([128, cfg.K81], BF16, tag="kt")
                dsum = spool.tile([128, 1], F32, tag="dsum")
                nc.scalar.activation(kt[:], t81[:], AF.Exp, accum_out=dsum[:])
                rd = rdpool.tile([128, 1], F32, tag="rd")
                nc.vector.reciprocal(rd[:], dsum[:])
                # write kt contiguously (128 descs of 162B), then load
                # the band via two XBAR transpose-DMAs: T[s, u] =
                # img[5815*rr + 9*u + s] with u = 10*w + g, so band col
                # layout is 640*rr + 10*w + g.
                dstap = AP(ximgs[t], 0, [[cfg.XST, 128], [1, cfg.K81]])
                nc.scalar.dma_start(out=dstap, in_=kt[:, :])
                band = bandpool.tile([128, 10 * 2 * W], BF16, tag="band")
                for r in range(2):
                    srcap = AP(ximgs[t], cfg.XRR * r, [[9, 10 * W], [1, 128]])
                    nc.sync.dma_start(
                        out=band[:, 640 * r : 640 * (r + 1)],
                        in_=srcap,
                        transpose=True,
                    )
                nc.vector.tensor_mul(band[0:72, :], band[0:72, :], bmsb[:])
                return band, rd

            def emit_svc(t, band, rd):
                ob = obpool.tile([128, 2 * 512], BF16, tag="ob")
                for cc in range(n_cc):
                    pv = pspool.tile([128, 512], F32, tag="ps")
                    bandv = band[0:72, :].rearrange(
                        "s (rr w g) -> s rr w g", rr=2, w=W
                    )
                    for gi in range(10):
                        nc.tensor.matmul(
                            pv[:, :],
                            bandv[:, :, :, gi],
                            ht_slice(2 * t + gi, cc),
                            start=(gi == 0),
                            stop=(gi == 9),
                        )
                    nc.scalar.activation(
                        ob[:, 512 * cc : 512 * (cc + 1)],
                        pv[:, :],
                        AF.Copy,
                        scale=rd[:, 0:1],
                    )
                nc.sync.dma_start(
                    out=out_e[t * 128 : (t + 1) * 128, :], in_=ob[:]
                )

            # ---- interleaved pair-granularity pipeline ----
            chains = {}

            def g3(t):
                chains[t] = emit_conv3(t)

            def svc(t):
                emit_svc(t, *chains.pop(t))

            n_c2b = cfg.OUT_ROWS // cfg.C2_ROWS
            if n_c2b == 4 and len(cfg.C1_BLOCKS) == 5:
                load_w1_chunk(0)
                nc.scalar.dma_start(out=b1sb[:], in_=b1_e[:, :])
                nc.scalar.dma_start(out=hmsb[:], in_=hmask_e[:, :])
                emit_c1_block(
                    0, after_inputs=lambda: [load_w1_chunk(c) for c in range(1, 8)]
                )
                emit_mask_top()
                emit_c1_block(1)
                emit_deferred_consts()
                for j in range(5):
                    need_ht2(j)
                emit_c2_block(0)
                g3(0); g3(1)
                emit_c1_block(2)
                g3(2); g3(3)
                emit_c2_block(1)
                g3(4); svc(0)
                g3(5); svc(1)
                emit_c1_block(3)
                g3(6); svc(2)
                g3(7); svc(3)
                emit_c2_block(2)
                g3(8); svc(4)
                g3(9); svc(5)
                emit_c1_block(4)
                emit_mask_bot()
                g3(10); svc(6)
                g3(11); svc(7)
                emit_c2_block(3)
                g3(12); svc(8)
                g3(13); svc(9)
                g3(14); svc(10)
                g3(15); svc(11)
                svc(12); svc(13); svc(14); svc(15)
            else:
                for ch in range(n_cin_ch):
                    load_w1_chunk(ch)
                nc.scalar.dma_start(out=b1sb[:], in_=b1_e[:, :])
                nc.scalar.dma_start(out=hmsb[:], in_=hmask_e[:, :])
                emit_deferred_consts()
                for j in range(5):
                    need_ht2(j)
                for bi in range(len(cfg.C1_BLOCKS)):
                    emit_c1_block(bi)
                emit_mask_top()
                emit_mask_bot()
                for b in range(n_c2b):
                    emit_c2_block(b)
                for t in range(PAIRS):
                    g3(t)
                for t in range(PAIRS):
                    svc(t)

    return nc


# ---------------- host side ----------------

_CACHED = None


def _get_graph():
    global _CACHED
    if _CACHED is None:
        _CACHED = build_graph(CFG)
        _CACHED.compile()
    return _CACHED


def make_band_mask(cfg):
    """Static validity mask for band tiles [WH, 1280], col = 640r+10w+g."""
    s = np.arange(cfg.WH)[:, None]
    col = np.arange(10 * 2 * cfg.W)[None, :]
    r = col // 640
    w = (col % 640) // 10
    g = col % 10
    dy = g - r
    dx = s - w
    m = (dy >= 0) & (dy <= 8) & (dx >= 0) & (dx <= 8)
    return m.astype(BF)


def shard_inputs(inputs, cfg):
    """Build per-core input maps from the full problem inputs."""
    cur = np.asarray(inputs["current_frame_low_features"])
    key = np.asarray(inputs["key_frame_low_features"])
    high = np.asarray(inputs["key_frame_high_features"])
    B, Cin, H, W = cur.shape

    w_reduce = np.asarray(inputs["w_reduce"])  # (CO1, Cin, 3, 3)
    w2 = np.asarray(inputs["w2"])  # (CO2, C2, 3, 3)
    w3 = np.asarray(inputs["w3"])  # (81, CO2, 1, 1)
    n_cin_ch = Cin // 128
    n_c2_ch = cfg.C2 // 128
    # w1 host layout [128ci, chunk, dy*m*co], Winograd-F(2,3) transformed
    G = np.array(
        [[1, 0, 0], [0.5, 0.5, 0.5], [0.5, -0.5, 0.5], [0, 0, 1]], np.float32
    )
    wr = w_reduce.reshape(cfg.CO1, n_cin_ch, 128, 3, 3)  # o c p y d
    w1h = np.ascontiguousarray(
        np.einsum("md,ocpyd->pcymo", G, wr).reshape(128, n_cin_ch, 12 * cfg.CO1)
    ).astype(BF)
    wr2 = w2.reshape(cfg.CO2, n_c2_ch, 128, 3, 3)  # o c p y d
    w2h = np.ascontiguousarray(
        np.einsum("md,ocpyd->pcymo", G, wr2).reshape(128, n_c2_ch, 12 * cfg.CO2)
    ).astype(BF)
    w3h = np.ascontiguousarray(
        w3.reshape(cfg.K81, cfg.CO2 // 128, 128).transpose(2, 1, 0)
    ).astype(BF)
    b1h = np.ascontiguousarray(
        np.asarray(inputs["b_reduce"]).reshape(cfg.CO1 // 128, 128).T
    ).astype(np.float32)
    b2h = np.ascontiguousarray(
        np.asarray(inputs["b2"]).reshape(cfg.CO2 // 128, 128).T
    ).astype(np.float32)
    b3h = np.broadcast_to(
        np.asarray(inputs["b3"]).astype(np.float32)[None, :], (128, cfg.K81)
    ).copy()
    bmask = make_band_mask(cfg)

    in_maps = []
    for core in range(B * cfg.HALVES):
        b, half = core // cfg.HALVES, core % cfg.HALVES
        s = half * cfg.OUT_ROWS
        # low features: rows [s-2, s+OUT_ROWS+2), w padded +-1, bf16,
        # layout [128, IN_ROWS, chunk*WP]
        lowpad = np.zeros((2, Cin, cfg.IN_ROWS, cfg.WP), np.float32)
        r0, r1 = s - 2, s + cfg.OUT_ROWS + 2
        cr0, cr1 = max(r0, 0), min(r1, H)
        lowpad[0, :, cr0 - r0 : cr1 - r0, 1 : 1 + W] = cur[b, :, cr0:cr1, :]
        lowpad[1, :, cr0 - r0 : cr1 - r0, 1 : 1 + W] = key[b, :, cr0:cr1, :]
        lowT = np.ascontiguousarray(
            lowpad.reshape(2, n_cin_ch, 128, cfg.IN_ROWS, cfg.WP // 2, 2)
            .transpose(0, 2, 3, 1, 5, 4)
        ).reshape(2, 128, cfg.IN_ROWS, n_cin_ch * cfg.WP).astype(BF)
        # high features: rows [s-4, s+OUT_ROWS+4), w padded +-4, transposed
        hp = np.zeros((cfg.HT_ROWS, cfg.WH, cfg.CH), np.float32)
        hr0, hr1 = s - 4, s + cfg.OUT_ROWS + 4
        chr0, chr1 = max(hr0, 0), min(hr1, H)
        hp[chr0 - hr0 : chr1 - hr0, 4 : 4 + W, :] = high[b, :, chr0:chr1, :].transpose(
            1, 2, 0
        )
        hmask = np.zeros((128, 2), np.float32)
        hmask[:, 0] = 0.0 if s == 0 else 1.0
        hmask[:, 1] = 0.0 if s + cfg.OUT_ROWS == H else 1.0
        in_maps.append(
            {
                "cur": lowT[0],
                "key": lowT[1],
                "highT": hp.astype(BF),
                "w1": w1h,
                "w2": w2h,
                "w3": w3h,
                "b1": b1h,
                "b2": b2h,
                "b3": b3h,
                "hmask": hmask,
                "bmask": bmask,
            }
        )
    return in_maps


def gather_outputs(results, cfg, H, W):
    out = np.zeros((cfg.B, cfg.CH, H, W), np.float32)
    for core, res in enumerate(results):
        b, half = core // cfg.HALVES, core % cfg.HALVES
        s = half * cfg.OUT_ROWS
        o = np.asarray(res["out"]).astype(np.float32).reshape(
            cfg.OUT_ROWS, W, cfg.CH
        )
        out[b, :, s : s + cfg.OUT_ROWS, :] = o.transpose(2, 0, 1)
    return out


def kernel(**inputs) -> np.ndarray:
    cfg = CFG
    nc = _get_graph()
    in_maps = shard_inputs(inputs, cfg)
    res = run_bass_kernel_spmd(nc, in_maps, core_ids=list(range(8)))
    return gather_outputs(res.results, cfg, cfg.H, cfg.W)

